# revision 37
# baseline (speedup 1.0000x reference)
"""Llama GQA attention layer (B=1, S=2048, E=4096, H=32, HKV=8, D=128) on 8
Trainium2 NeuronCores.

Sharding: tensor-parallel over heads. Core c owns Q heads 4c..4c+3 and KV head
c (KV groups stay intact), plus the matching Wo input-dim slice. Each core
computes a full [S, E] partial of the o_proj output; the host sums the 8
partials (the "all-reduce after o_proj").

All matmul operands are bf16 (PSUM accumulation fp32). The PE is the
bottleneck engine (~92% busy at 2.4 GHz), so everything that is not an
irreducible matmul is moved off it, and the PE stream is braided so it
never waits on the scalar engine:

  QKV: k and v chains interleaved chunk-by-chunk (DMA-paced at startup),
    q chains after. RoPE: half-rotation via two SBUF->SBUF partition-offset
    DMAs (off the critical path; sign folded into the sin table, 1/sqrt(D)
    folded into q's cos/sin tables), then 3 DVE 2x-mode ops. v is
    transposed to [tok, d] via PE transpose, banks alternating so copies
    overlap transposes.
  attention: flat software-pipelined loop over (head, key-tile); scoresT[k,q]
    -> exp on ScalarE -> av accumulates in PSUM. The softmax denominator is
    NOT a per-tile PE matmul (that would cost 69k PE cycles): exp tiles are
    accumulated elementwise in bf16 on the DVE (den error ~0.1% after the
    final exact 128-way matmul sum) and a single ones-matmul per (head,
    group) broadcasts the denominator into every PSUM row. The causal
    diagonal mask is added by the PE itself: a 128-col matmul (cmask^T
    stationary x identity moving) appended to the score accumulation.
    Softmax has no max subtraction (scores ~ N(0,1), exp cannot overflow).
    Epilogue: reciprocal_approx_fast (DVE) + one DVE mul -> ao.
  braid: after den-removal the attention inner loop is scalar-exp-paced
    (~530ns/iter vs ~390ns PE), so o_proj matmuls of the PREVIOUS group are
    braided into the attention loop one per iteration (eg-pair structure on
    2 PSUM banks); group 0 braids the tail q-chains of its own QKV instead
    (8 chunks/iter so every qro head is roped before its scores are emitted
    - emission-order deadlock otherwise). QKV(g+1) runs after attention(g).
  o_proj outside the braid runs baseline-wide: all 8 PSUM banks accumulate
    over the 4 local heads, so drains (alternating scalar/DVE) never stall
    the PE; per-token-tile output DMAed bf16, the last tile in quarters so
    the final DMA is small. Host sums the 8 partial [S, E] outputs.

PSUM banks: SP0-2 (score rotation / k,v chains / vtr), AV0-1 (av by head
parity), DN (den broadcast / rope swap / vtr), OA-OB (o_proj pairs /
q chains) = 8; wide o_proj uses all 8 after attention retires.

DMA: all host-side arrays are packed so every descriptor moves long
contiguous per-partition lines (hsT blocks [128, 8*512], weights
[128, NE*D]); first pieces are small so the first matmul fires early;
hst(g+1) block prefetches are staggered inside attention(g) at milestones
that respect the 5-buffer rotation against the braided q-chain reads.
"""

import sys
import types

if "/opt/trn_rl_repo" not in sys.path:
    sys.path.insert(0, "/opt/trn_rl_repo")

import numpy as np
import ml_dtypes

import concourse.bass as bass
import concourse.tile as tile
from concourse import bacc, mybir
from concourse.bass_utils import run_bass_kernel_spmd

F32 = mybir.dt.float32
BF16 = mybir.dt.bfloat16
EXP = mybir.ActivationFunctionType.Exp
BF = ml_dtypes.bfloat16

S = 2048
E = 4096
H = 32
HKV = 8
D = 128
NCORES = 8
HL = H // NCORES          # 4 local q heads per core
TG = 512                  # token group (moving-dim tile)
NG = S // TG              # 4 token groups
NE = E // 128             # 32 contraction chunks
NK = S // 128             # 16 key tiles
EB = 8                    # e-chunks per hsT DMA block
NB = NE // EB             # 4 blocks per group
NEG = -1e9

TRACE = [False]
LAST_EXEC_NS = [None]
LAST_RES = [None]

_PROGRAMS = {}


def _install_ntff_hook():
    if "antenv.axon_hooks" in sys.modules:
        return
    mod = types.ModuleType("antenv.axon_hooks")
    hook = [None]
    mod.set_axon_ntff_profile_hook = lambda h: hook.__setitem__(0, h)
    mod.get_axon_ntff_profile_hook = lambda: hook[0]
    sys.modules["antenv.axon_hooks"] = mod
    try:
        from trn_agent_boot.trn_boot import _ntff_profile_via_ctypes

        mod.set_axon_ntff_profile_hook(
            _ntff_profile_via_ctypes("/opt/axon/libaxon_pjrt.so"))
    except Exception:
        pass


def set_trace(on=True):
    if on:
        _install_ntff_hook()
    TRACE[0] = on


def _build_program(mode):
    """mode: 'causal' (skip above-diagonal key tiles, live-column diagonal),
    'full' (no mask), 'general' (additive mask streamed from DRAM).
    """
    nc = bacc.Bacc(trn_type="TRN2", target_bir_lowering=False, debug=False)

    # packed DRAM layouts: per-partition lines fully contiguous
    hsT_d = nc.dram_tensor("hsT", [NG, NB, 128, EB, TG], BF16,
                           kind="ExternalInput").ap()
    wqT_d = nc.dram_tensor("wqT", [HL, 128, NE, D], BF16,
                           kind="ExternalInput").ap()
    wkT_d = nc.dram_tensor("wkT", [128, NE, D], BF16,
                           kind="ExternalInput").ap()
    wvT_d = nc.dram_tensor("wvT", [128, NE, D], BF16,
                           kind="ExternalInput").ap()
    woT_d = nc.dram_tensor("woT", [128, HL, E], BF16,
                           kind="ExternalInput").ap()
    cosq_d = nc.dram_tensor("cosq", [D, S], BF16, kind="ExternalInput").ap()
    sinq_d = nc.dram_tensor("sinq", [D, S], BF16, kind="ExternalInput").ap()
    cosk_d = nc.dram_tensor("cosk", [D, S], BF16, kind="ExternalInput").ap()
    sink_d = nc.dram_tensor("sink", [D, S], BF16, kind="ExternalInput").ap()
    onesm_d = nc.dram_tensor("onesm", [128, 128], BF16,
                             kind="ExternalInput").ap()
    identb_d = nc.dram_tensor("identb", [128, 128], BF16,
                              kind="ExternalInput").ap()
    ident_d = nc.dram_tensor("ident", [128, 128], F32,
                             kind="ExternalInput").ap()
    if mode == "causal":
        cmaskT_d = nc.dram_tensor("cmaskT", [128, 128], BF16,
                                  kind="ExternalInput").ap()
    elif mode == "general":
        maskT_d = nc.dram_tensor("maskT", [S, S], BF16,
                                 kind="ExternalInput").ap()
    outp_d = nc.dram_tensor("outp", [NK, 128, E], BF16,
                            kind="ExternalOutput").ap()

    with tile.TileContext(nc) as tc:
      with nc.allow_low_precision(reason="bf16 attention kernel"), \
           tc.tile_pool(name="const", bufs=1) as cp, \
           tc.tile_pool(name="persist", bufs=1) as pp, \
           tc.tile_pool(name="hstp", bufs=5) as hst_pool, \
           tc.tile_pool(name="qrop", bufs=2) as qro_pool, \
           tc.tile_pool(name="aop", bufs=2) as ao_pool, \
           tc.tile_pool(name="xsp", bufs=2) as xs_pool, \
           tc.tile_pool(name="swp", bufs=2) as sw_pool, \
           tc.tile_pool(name="t12p", bufs=2) as t12_pool, \
           tc.tile_pool(name="vsp", bufs=1) as vs_pool, \
           tc.tile_pool(name="accp", bufs=1) as acc_pool, \
           tc.tile_pool(name="exp", bufs=5) as ex_pool, \
           tc.tile_pool(name="rcp", bufs=2) as rc_pool, \
           tc.tile_pool(name="ostp", bufs=2) as ost_pool, \
           tc.tile_pool(name="mtp", bufs=4) as mt_pool, \
           tc.tile_pool(name="ps", bufs=1, space="PSUM") as ps:

        # --- persistent SBUF ---
        wq_sb = pp.tile([128, HL, NE, D], BF16, name="wq_sb")
        wk_sb = pp.tile([128, NE, D], BF16, name="wk_sb")
        wv_sb = pp.tile([128, NE, D], BF16, name="wv_sb")
        wo_sb = pp.tile([128, HL, E], BF16, name="wo_sb")
        cq_sb = pp.tile([128, S], BF16, name="cq_sb")
        sq_sb = pp.tile([128, S], BF16, name="sq_sb")
        ck_sb = pp.tile([128, S], BF16, name="ck_sb")
        sk_sb = pp.tile([128, S], BF16, name="sk_sb")
        krope = pp.tile([128, S], BF16, name="krope")
        vnat = pp.tile([128, NK, D], BF16, name="vnat")
        ones_sb = cp.tile([128, 128], BF16, name="ones_sb")
        identb = cp.tile([128, 128], BF16, name="identb")
        ident = cp.tile([128, 128], F32, name="ident")
        if mode == "causal":
            cmaskT = cp.tile([128, 128], BF16, name="cmaskT")

        hst = {}

        def issue_hst(g, b):
            t = hst_pool.tile([128, EB, TG], BF16, tag="hst",
                              name=f"hst{g}_{b}")
            nc.sync.dma_start(out=t, in_=hsT_d[g, b])
            hst[(g, b)] = t

        def hs_chunk(g, e):
            return hst[(g, e // EB)][:, e % EB, :]

        # --- startup DMAs on the sync hardware queue, ordered along the
        # k/v chain consumption order; first pieces are small so the first
        # matmul fires as early as possible. ---
        h0 = {}
        for b in range(NB):
            h0[b] = hst_pool.tile([128, EB, TG], BF16, tag="hst",
                                  name=f"hst0_{b}")
            hst[(0, b)] = h0[b]

        nc.sync.dma_start(out=wk_sb[:, 0:2, :], in_=wkT_d[:, 0:2, :])
        nc.sync.dma_start(out=h0[0][:, 0:2, :], in_=hsT_d[0, 0][:, 0:2, :])
        nc.sync.dma_start(out=wv_sb[:, 0:2, :], in_=wvT_d[:, 0:2, :])
        nc.sync.dma_start(out=wk_sb[:, 2:8, :], in_=wkT_d[:, 2:8, :])
        nc.sync.dma_start(out=h0[0][:, 2:8, :], in_=hsT_d[0, 0][:, 2:8, :])
        nc.sync.dma_start(out=wv_sb[:, 2:8, :], in_=wvT_d[:, 2:8, :])
        nc.sync.dma_start(out=wk_sb[:, 8:20, :], in_=wkT_d[:, 8:20, :])
        nc.sync.dma_start(out=h0[1], in_=hsT_d[0, 1])
        nc.sync.dma_start(out=wv_sb[:, 8:20, :], in_=wvT_d[:, 8:20, :])
        nc.sync.dma_start(out=wk_sb[:, 20:32, :], in_=wkT_d[:, 20:32, :])
        nc.sync.dma_start(out=h0[2], in_=hsT_d[0, 2])
        nc.sync.dma_start(out=wv_sb[:, 20:32, :], in_=wvT_d[:, 20:32, :])
        nc.sync.dma_start(out=h0[3], in_=hsT_d[0, 3])
        nc.sync.dma_start(out=ident, in_=ident_d)
        # rope tables stream in lazily: only group 0's slices at startup
        # (saves 1.5MB off the startup-critical DMA window); later slices
        # are prefetched inside attention(g) alongside the hst blocks
        nc.sync.dma_start(out=ck_sb[:, 0:TG], in_=cosk_d[:, 0:TG])
        nc.sync.dma_start(out=sk_sb[:, 0:TG], in_=sink_d[:, 0:TG])
        nc.sync.dma_start(out=wq_sb[:, 0], in_=wqT_d[0])
        nc.sync.dma_start(out=cq_sb[:, 0:TG], in_=cosq_d[:, 0:TG])
        nc.sync.dma_start(out=sq_sb[:, 0:TG], in_=sinq_d[:, 0:TG])
        nc.sync.dma_start(out=wq_sb[:, 1], in_=wqT_d[1])
        nc.sync.dma_start(out=identb, in_=identb_d)
        if mode == "causal":
            nc.sync.dma_start(out=cmaskT, in_=cmaskT_d)
        nc.sync.dma_start(out=wq_sb[:, 2], in_=wqT_d[2])
        nc.sync.dma_start(out=wq_sb[:, 3], in_=wqT_d[3])
        nc.sync.dma_start(out=ones_sb, in_=onesm_d)
        nc.sync.dma_start(out=wo_sb[:, :, :E // 2], in_=woT_d[:, :, :E // 2])
        nc.sync.dma_start(out=wo_sb[:, :, E // 2:], in_=woT_d[:, :, E // 2:])

        # --- helpers ---

        def rope_dve(x_ps, cos_sb, sin_sb, g, out_ap):
            """out = x*cos + rot_half(x)*sin; the half-rotation is two
            SBUF->SBUF partition-offset DMAs (DVE ops cannot cross partition
            offsets - the BIR verifier requires same start partitions - and
            a PE permutation matmul would cost 512 PE cycles per rope).
            Rope latency is off the critical path (the consuming scores run
            several microseconds later). Sign of the rotation is folded into
            sin_sb's first half; all operands stay bf16 SBUF so the DVE runs
            in 2x mode."""
            t0 = g * TG
            xs = xs_pool.tile([128, TG], BF16, tag="xs", name="xs")
            nc.scalar.copy(out=xs, in_=x_ps)
            sw = sw_pool.tile([128, TG], BF16, tag="sw", name="sw")
            nc.sync.dma_start(out=sw[0:64], in_=xs[64:128])
            nc.sync.dma_start(out=sw[64:128], in_=xs[0:64])
            t1 = t12_pool.tile([128, TG], BF16, tag="t1", name="t1")
            t2 = t12_pool.tile([128, TG], BF16, tag="t2", name="t2")
            nc.vector.tensor_mul(t1, xs, cos_sb[:, t0:t0 + TG])
            nc.vector.tensor_mul(t2, sw, sin_sb[:, t0:t0 + TG])
            nc.vector.tensor_add(out_ap, t1, t2)

        def kv_chains(g):
            """k and v chains interleaved chunk-by-chunk, then k-rope and
            the v transpose into vnat."""
            t0 = g * TG
            k_ps = ps.tile([128, TG], F32, tag="SP0", name=f"k_ps{g}")
            v_ps = ps.tile([128, TG], F32, tag="SP1", name=f"v_ps{g}")
            for e in range(NE):
                he = hs_chunk(g, e)
                nc.tensor.matmul(k_ps, wk_sb[:, e, :], he,
                                 start=(e == 0), stop=(e == NE - 1))
                nc.tensor.matmul(v_ps, wv_sb[:, e, :], he,
                                 start=(e == 0), stop=(e == NE - 1))
            rope_dve(k_ps, ck_sb, sk_sb, g, krope[:, t0:t0 + TG])
            vs = vs_pool.tile([128, TG], F32, tag="vs", name="vs")
            nc.scalar.copy(out=vs, in_=v_ps)
            for j in range(4):
                # alternate banks so transpose j+1 overlaps copy j
                tr = ps.tile([128, 128], F32, tag=("DN", "SP2")[j % 2],
                             name="tr")
                nc.tensor.transpose(tr, vs[:, 128 * j:128 * (j + 1)], ident)
                nc.vector.tensor_copy(vnat[:, 4 * g + j, :], tr)

        def qchain_units(g, f, tag, qro_t):
            """One emission callable per e-chunk of q-head f's chain; the
            last one also emits the rope."""
            holder = {}

            def mk(e):
                def emit():
                    if e == 0:
                        holder["ps"] = ps.tile([128, TG], F32, tag=tag,
                                               name=f"q_ps{g}_{f}")
                    nc.tensor.matmul(holder["ps"], wq_sb[:, f, e, :],
                                     hs_chunk(g, e),
                                     start=(e == 0), stop=(e == NE - 1))
                    if e == NE - 1:
                        rope_dve(holder["ps"], cq_sb, sq_sb, g,
                                 qro_t[:, f, :])
                return emit

            return [mk(e) for e in range(NE)]

        ao_tiles = {}
        qro_tiles = {}

        def oproj_pairs(gg, tis):
            """Yield one emission callable per o_proj matmul of group gg,
            token tiles `tis` only (braided as filler into attention(gg+1)).
            eg pairs use the OA/OB PSUM banks, accumulating over the 4 local
            heads; drains alternate scalar/DVE; half-row DMAs ride along."""
            ao_t = ao_tiles[gg]
            for ti in tis:
                ost = ost_pool.tile([128, E], BF16, tag="ost",
                                    name=f"ost{gg}_{ti}")
                state = {}
                for pair in range(4):
                    for h in range(HL):
                        for sub in range(2):
                            def emit(pair=pair, h=h, sub=sub, ti=ti,
                                     ost=ost, state=state):
                                eg = 2 * pair + sub
                                if h == 0:
                                    state[sub] = ps.tile(
                                        [128, TG], F32,
                                        tag="OA" if sub == 0 else "OB",
                                        name=f"op{gg}_{ti}_{eg}")
                                nc.tensor.matmul(
                                    state[sub],
                                    ao_t[:, h, 128 * ti:128 * (ti + 1)],
                                    wo_sb[:, h, TG * eg:TG * (eg + 1)],
                                    start=(h == 0), stop=(h == HL - 1))
                                if h == HL - 1 and sub == 0:
                                    nc.scalar.copy(
                                        out=ost[:, TG * eg:TG * (eg + 1)],
                                        in_=state[sub])
                                elif h == HL - 1:
                                    nc.vector.tensor_copy(
                                        ost[:, TG * eg:TG * (eg + 1)],
                                        state[sub])
                                    ro = 4 * gg + ti
                                    if pair == 1:
                                        nc.sync.dma_start(
                                            out=outp_d[ro][:, :E // 2],
                                            in_=ost[:, :E // 2])
                                    elif pair == 3:
                                        nc.sync.dma_start(
                                            out=outp_d[ro][:, E // 2:],
                                            in_=ost[:, E // 2:])
                            yield emit

        OTAGS = ["SP0", "SP1", "SP2", "AV0", "AV1", "DN", "OA", "OB"]

        def oproj_wide(gg, tis, last_group):
            """Baseline-style o_proj for token tiles `tis`: all 8 PSUM banks
            accumulate over the 4 local heads (ao stationary loaded once per
            (ti, h)), so drains never stall the PE. Runs only outside the
            braid (after attention, when the attention banks are free). The
            very last tile drains under the h3 sweep and DMAs out in
            quarters so the final DMA is small."""
            ao_t = ao_tiles[gg]
            for ti in tis:
                ost = ost_pool.tile([128, E], BF16, tag="ost",
                                    name=f"ost{gg}_{ti}")
                last = last_group and ti == 3
                ops = {}

                def drain_eg(eg, ost=ost, ti=ti, last=last):
                    dst = ost[:, TG * eg:TG * (eg + 1)]
                    if eg % 2:
                        nc.vector.tensor_copy(dst, ops[eg])
                    else:
                        nc.scalar.copy(out=dst, in_=ops[eg])
                    ro = 4 * gg + ti
                    if last:
                        # eighth-granular so the final transfer is tiny
                        nc.sync.dma_start(
                            out=outp_d[ro][:, TG * eg:TG * (eg + 1)],
                            in_=ost[:, TG * eg:TG * (eg + 1)])
                    else:
                        if eg == 3:
                            nc.sync.dma_start(out=outp_d[ro][:, :E // 2],
                                              in_=ost[:, :E // 2])
                        elif eg == 7:
                            nc.sync.dma_start(out=outp_d[ro][:, E // 2:],
                                              in_=ost[:, E // 2:])

                for h in range(HL):
                    for eg in range(E // TG):
                        if h == 0:
                            ops[eg] = ps.tile([128, TG], F32, tag=OTAGS[eg],
                                              name=f"opw{eg}")
                        nc.tensor.matmul(
                            ops[eg], ao_t[:, h, 128 * ti:128 * (ti + 1)],
                            wo_sb[:, h, TG * eg:TG * (eg + 1)],
                            start=(h == 0), stop=(h == HL - 1))
                        if last and h == HL - 1:
                            drain_eg(eg)
                if not last:
                    for eg in range(E // TG):
                        drain_eg(eg)

        def attention(g, fillers):
            """Flat pipelined (head, key-tile) loop; pulls one filler PE-op
            per iteration; issues hst(g+1) prefetches at milestones."""
            t0 = g * TG
            nk = 4 * g + 4 if mode == "causal" else NK
            qro_t = qro_tiles[g]
            ao_t = ao_tiles[g]
            av_ps = {}
            acc = {}
            queue = []
            srot = [0]

            def front(h, ki):
                c0 = 128 * (ki - 4 * g) if (mode == "causal" and ki >= 4 * g) \
                    else 0
                sp = ps.tile([128, TG], F32, tag=f"SP{srot[0] % 3}",
                             name="sp")
                srot[0] += 1
                diag = mode == "causal" and ki >= 4 * g
                nc.tensor.matmul(sp[:, c0:],
                                 krope[:, 128 * ki:128 * (ki + 1)],
                                 qro_t[:, h, c0:], start=True,
                                 stop=not diag, skip_group_check=True)
                if diag:
                    # += cmask via PE: cmask^T stationary x identity moving
                    nc.tensor.matmul(sp[:, c0:c0 + 128], cmaskT, identb,
                                     start=False, stop=True,
                                     skip_group_check=True)
                elif mode == "general":
                    mt = mt_pool.tile([128, TG], BF16, tag="mt", name="mt")
                    nc.sync.dma_start(
                        out=mt,
                        in_=maskT_d[128 * ki:128 * (ki + 1), t0:t0 + TG])
                    nc.vector.tensor_add(sp, sp, mt)
                ex = ex_pool.tile([128, TG], BF16, tag="ex", name="ex")
                nc.scalar.activation(out=ex[:, c0:], in_=sp[:, c0:], func=EXP)
                return (h, ki, c0, ex)

            def drain_one():
                h, ki, c0, ex = queue.pop(0)
                if ki == 0:
                    av_ps[h] = ps.tile([128, TG], F32, tag=f"AV{h % 2}",
                                       name=f"av{h}")
                    acc[h] = acc_pool.tile([128, TG], BF16, tag=f"acc{h}",
                                           name=f"acc{g}_{h}")
                last = (ki == nk - 1)
                nc.tensor.matmul(av_ps[h][:, c0:], vnat[:, ki, :],
                                 ex[:, c0:], start=(ki == 0), stop=last,
                                 skip_group_check=True)
                if ki == 0:
                    nc.vector.tensor_copy(acc[h], ex)
                else:
                    nc.vector.tensor_add(acc[h][:, c0:], acc[h][:, c0:],
                                         ex[:, c0:])
                if last:
                    den = ps.tile([128, TG], F32, tag="DN", name=f"den{h}")
                    nc.tensor.matmul(den, ones_sb, acc[h],
                                     start=True, stop=True)
                    rc = rc_pool.tile([128, TG], F32, tag="rc", name="rc")
                    nc.vector.reciprocal_approx_fast(out=rc, in_=den)
                    nc.vector.tensor_mul(ao_t[:, h, :], av_ps[h], rc)

            # hst(g+1) prefetch milestones. g=0 is special: the braided
            # q2/q3 chains still READ hst(0,*) buffers during early
            # iterations; each block's DMA must be emitted only after the
            # braided reads of the buffer it reuses (5-buffer rotation).
            miles = {1: 0, 6: 1, 7: 2, 8: 3} if g == 0 else \
                    {1: 0, 5: 1, 9: 2, 13: 3}
            nev = 0
            for h in range(HL):
                for ki in range(nk):
                    queue.append(front(h, ki))
                    nev += 1
                    if g < NG - 1 and nev in miles:
                        issue_hst(g + 1, miles[nev])
                    if g < NG - 1 and nev == 2:
                        t1g = (g + 1) * TG
                        for tb_sb, tb_d in ((ck_sb, cosk_d), (sk_sb, sink_d),
                                            (cq_sb, cosq_d), (sq_sb, sinq_d)):
                            nc.sync.dma_start(
                                out=tb_sb[:, t1g:t1g + TG],
                                in_=tb_d[:, t1g:t1g + TG])
                    for fl in fillers:
                        fl()
                        break
                    if len(queue) >= 3:
                        drain_one()
            while queue:
                drain_one()

        # ---- prologue: QKV(0); q2/q3 braid into attn(0) ----
        qro_tiles[0] = qro_pool.tile([128, HL, TG], BF16, tag="qro",
                                     name="qro0")
        kv_chains(0)
        for f in (0, 1):
            for u in qchain_units(0, f, ("OA", "OB")[f], qro_tiles[0]):
                u()

        def prologue_filler_gen():
            # 8 chain chunks per pull: q2 fully roped by iteration 3 (h=0),
            # q3 by iteration 7 (h=1) - both well before their heads' scores
            units = (qchain_units(0, 2, "OA", qro_tiles[0])
                     + qchain_units(0, 3, "OB", qro_tiles[0]))
            for i in range(0, len(units), 8):
                def emit8(i=i):
                    for u in units[i:i + 8]:
                        u()
                yield emit8

        # ---- supersteps ----
        # braided tis of oproj(g-1): just enough pair-mode units to cover
        # every attention(g) iteration (32 units per ti); the rest runs
        # wide (8-bank) right after attention, stall-free.
        BRAID_TIS = {1: (0,), 2: (0, 1), 3: (0, 1)}
        for g in range(NG):
            ao_tiles[g] = ao_pool.tile([128, HL, TG], BF16, tag="ao",
                                       name=f"ao{g}")
            if g == 0:
                fillers = prologue_filler_gen()
            else:
                fillers = oproj_pairs(g - 1, BRAID_TIS[g])
            attention(g, fillers)
            for fl in fillers:
                fl()
            if g >= 1:
                rest = tuple(t for t in range(4) if t not in BRAID_TIS[g])
                oproj_wide(g - 1, rest, last_group=False)
            if g < NG - 1:
                kv_chains(g + 1)
                qro_tiles[g + 1] = qro_pool.tile([128, HL, TG], BF16,
                                                 tag="qro", name=f"qro{g+1}")
                for f in range(HL):
                    for u in qchain_units(g + 1, f, ("OA", "OB")[f % 2],
                                          qro_tiles[g + 1]):
                        u()

        # ---- coda: o_proj of the last group, all wide ----
        oproj_wide(NG - 1, (0, 1, 2, 3), last_group=True)

    nc.compile()
    return nc


_CONSTS = None


def _consts():
    global _CONSTS
    if _CONSTS is None:
        kp = np.arange(128)[:, None]
        qc = np.arange(128)[None, :]
        cmask = np.where(qc >= kp, 0.0, NEG).astype(np.float32)
        cmaskT = np.ascontiguousarray(cmask.T).astype(BF)
        ones = np.ones((128, 128), dtype=BF)
        identb = np.eye(128, dtype=np.float32).astype(BF)
        ident = np.eye(128, dtype=np.float32)
        _CONSTS = (cmaskT, ones, identb, ident)
    return _CONSTS


def _rope_tables(position_ids):
    pos = np.asarray(position_ids[0]).astype(np.float32)          # [S]
    inv_freq = (1.0 / (10000.0 ** (np.arange(0, D, 2, dtype=np.float32) / D)))
    freqs = pos[:, None] * inv_freq[None, :]                      # [S, 64]
    emb = np.concatenate([freqs, freqs], axis=1)                  # [S, 128]
    cosT = np.cos(emb).T.astype(np.float32).copy()                # [128, S]
    sinT = np.sin(emb).T.astype(np.float32)
    sinflipT = np.concatenate([-sinT[:64], sinT[64:]], axis=0)
    sc = np.float32(1.0 / np.sqrt(D))
    return ((cosT * sc).astype(BF), (sinflipT * sc).astype(BF),
            cosT.astype(BF), sinflipT.astype(BF))


def _pack_head(wT):
    """[E, dout] -> [128, NE, dout] with partition = within-chunk row."""
    dout = wT.shape[1]
    return np.ascontiguousarray(
        wT.reshape(NE, 128, dout).transpose(1, 0, 2)).astype(BF)


def kernel(hidden_states, position_ids, attention_mask, Wq, Wk, Wv, Wo):
    hidden_states = np.asarray(hidden_states)
    B = hidden_states.shape[0]
    assert hidden_states.shape == (B, S, E), hidden_states.shape
    assert B == 1

    mask = np.asarray(attention_mask, dtype=np.float32)[0, 0]
    if not mask.any():
        mode = "full"
    elif np.array_equal(mask, np.triu(np.full((S, S), NEG, dtype=np.float32), 1)):
        mode = "causal"
    else:
        mode = "general"

    if mode not in _PROGRAMS:
        _PROGRAMS[mode] = _build_program(mode)
    nc = _PROGRAMS[mode]

    hs = np.asarray(hidden_states[0], dtype=np.float32)
    # [E, S] -> packed blocks [NG, NB, 128, EB, TG]: E factored as
    # (NB, EB, 128) chunk-major (e = c*128 + p), S as (NG, TG); every
    # per-partition DMA line is then EB*TG*2 = 8KB contiguous.
    hsT = np.ascontiguousarray(
        hs.T.reshape(NB, EB, 128, NG, TG).transpose(3, 0, 2, 1, 4)).astype(BF)
    cosq, sinq, cosk, sink = _rope_tables(np.asarray(position_ids))
    Wq = np.asarray(Wq, dtype=np.float32)
    Wk = np.asarray(Wk, dtype=np.float32)
    Wv = np.asarray(Wv, dtype=np.float32)
    Wo = np.asarray(Wo, dtype=np.float32)
    cmaskT, ones, identb, ident = _consts()

    in_maps = []
    for c in range(NCORES):
        wq_c = Wq[512 * c:512 * (c + 1), :]            # [512, E] head-major
        wq_pack = np.stack([_pack_head(wq_c[128 * h:128 * (h + 1), :].T)
                            for h in range(HL)])       # [HL, 128, NE, D]
        wk_pack = _pack_head(Wk[128 * c:128 * (c + 1), :].T)
        wv_pack = _pack_head(Wv[128 * c:128 * (c + 1), :].T)
        # wo: [E, 512] slice -> [512, E] -> [HL, 128, E] -> [128, HL, E]
        wo_pack = np.ascontiguousarray(
            Wo[:, 512 * c:512 * (c + 1)].T.reshape(HL, 128, E)
            .transpose(1, 0, 2)).astype(BF)
        m = {
            "hsT": hsT,
            "wqT": wq_pack,
            "wkT": wk_pack,
            "wvT": wv_pack,
            "woT": wo_pack,
            "cosq": cosq, "sinq": sinq, "cosk": cosk, "sink": sink,
            "onesm": ones, "identb": identb, "ident": ident,
        }
        if mode == "causal":
            m["cmaskT"] = cmaskT
        elif mode == "general":
            m["maskT"] = np.ascontiguousarray(mask.T).astype(BF)
        in_maps.append(m)

    res = run_bass_kernel_spmd(nc, in_maps, core_ids=list(range(NCORES)),
                               trace=TRACE[0])
    LAST_EXEC_NS[0] = res.exec_time_ns
    LAST_RES[0] = res

    acc = np.zeros((NK, 128, E), dtype=np.float32)
    for c in range(NCORES):
        acc += res.results[c]["outp"].astype(np.float32)
    out = acc.reshape(S, E)
    return out[None, :, :]


# revision 40
# speedup vs baseline: 1.0284x; 1.0284x over previous
"""Llama GQA attention layer (B=1, S=2048, E=4096, H=32, HKV=8, D=128) on 8
Trainium2 NeuronCores.

Sharding: tensor-parallel over heads. Core c owns Q heads 4c..4c+3 and KV head
c (KV groups stay intact), plus the matching Wo input-dim slice. Each core
computes a full [S, E] partial of the o_proj output; the host sums the 8
partials (the "all-reduce after o_proj").

All matmul operands are bf16 (PSUM accumulation fp32). The PE is the
bottleneck engine (~92% busy at 2.4 GHz), so everything that is not an
irreducible matmul is moved off it, and the PE stream is braided so it
never waits on the scalar engine:

  QKV: k and v chains interleaved chunk-by-chunk (DMA-paced at startup),
    q chains after. RoPE: half-rotation via a PE permutation matmul (sign
    folded into the sin table, 1/sqrt(D) folded into q's cos/sin tables;
    a DVE rotate is illegal - same-start-partition rule - and an SBUF DMA
    rotate loses more to sync-queue contention than the 512 PE cycles),
    then 3 DVE ops. v is transposed to [tok, d] via PE transpose, banks
    alternating so copies overlap transposes.
  attention: flat software-pipelined loop over (head, key-tile); scoresT[k,q]
    -> exp on ScalarE -> av accumulates in PSUM. The softmax denominator is
    NOT a per-tile PE matmul (that would cost 69k PE cycles): exp tiles are
    accumulated elementwise in bf16 on the DVE (den error ~0.1% after the
    final exact 128-way matmul sum) and a single ones-matmul per (head,
    group) broadcasts the denominator into every PSUM row. The causal
    diagonal mask is added by the PE itself: a 128-col matmul (cmask^T
    stationary x identity moving) appended to the score accumulation.
    Softmax has no max subtraction (scores ~ N(0,1), exp cannot overflow).
    Epilogue: reciprocal_approx_fast (DVE) + one DVE mul -> ao.
  braid: after den-removal the attention inner loop is scalar-exp-paced
    (~530ns/iter vs ~390ns PE), so o_proj matmuls of the PREVIOUS group are
    braided into the attention loop one per iteration (eg-pair structure on
    2 PSUM banks); group 0 braids the tail q-chains of its own QKV instead
    (8 chunks/iter so every qro head is roped before its scores are emitted
    - emission-order deadlock otherwise). QKV(g+1) runs after attention(g).
  o_proj outside the braid runs baseline-wide: all 8 PSUM banks accumulate
    over the 4 local heads, so drains (alternating scalar/DVE) never stall
    the PE; per-token-tile output DMAed bf16, the last tile in quarters so
    the final DMA is small. Host sums the 8 partial [S, E] outputs.

PSUM banks: SP0-2 (score rotation / k,v chains / vtr), AV0-1 (av by head
parity), DN (den broadcast / rope swap / vtr), OA-OB (o_proj pairs /
q chains) = 8; wide o_proj uses all 8 after attention retires.

DMA: all host-side arrays are packed so every descriptor moves long
contiguous per-partition lines (hsT blocks [128, 8*512], weights
[128, NE*D]); first pieces are small so the first matmul fires early;
hst(g+1) block prefetches are staggered inside attention(g) at milestones
that respect the 5-buffer rotation against the braided q-chain reads.
"""

import sys
import types

if "/opt/trn_rl_repo" not in sys.path:
    sys.path.insert(0, "/opt/trn_rl_repo")

import numpy as np
import ml_dtypes

import concourse.bass as bass
import concourse.tile as tile
from concourse import bacc, mybir
from concourse.bass_utils import run_bass_kernel_spmd

F32 = mybir.dt.float32
BF16 = mybir.dt.bfloat16
EXP = mybir.ActivationFunctionType.Exp
BF = ml_dtypes.bfloat16

S = 2048
E = 4096
H = 32
HKV = 8
D = 128
NCORES = 8
HL = H // NCORES          # 4 local q heads per core
TG = 512                  # token group (moving-dim tile)
NG = S // TG              # 4 token groups
NE = E // 128             # 32 contraction chunks
NK = S // 128             # 16 key tiles
EB = 8                    # e-chunks per hsT DMA block
NB = NE // EB             # 4 blocks per group
NEG = -1e9

TRACE = [False]
LAST_EXEC_NS = [None]
LAST_RES = [None]

_PROGRAMS = {}


def _install_ntff_hook():
    if "antenv.axon_hooks" in sys.modules:
        return
    mod = types.ModuleType("antenv.axon_hooks")
    hook = [None]
    mod.set_axon_ntff_profile_hook = lambda h: hook.__setitem__(0, h)
    mod.get_axon_ntff_profile_hook = lambda: hook[0]
    sys.modules["antenv.axon_hooks"] = mod
    try:
        from trn_agent_boot.trn_boot import _ntff_profile_via_ctypes

        mod.set_axon_ntff_profile_hook(
            _ntff_profile_via_ctypes("/opt/axon/libaxon_pjrt.so"))
    except Exception:
        pass


def set_trace(on=True):
    if on:
        _install_ntff_hook()
    TRACE[0] = on


def _build_program(mode):
    """mode: 'causal' (skip above-diagonal key tiles, live-column diagonal),
    'full' (no mask), 'general' (additive mask streamed from DRAM).
    """
    nc = bacc.Bacc(trn_type="TRN2", target_bir_lowering=False, debug=False)

    # packed DRAM layouts: per-partition lines fully contiguous
    hsT_d = nc.dram_tensor("hsT", [NG, NB, 128, EB, TG], BF16,
                           kind="ExternalInput").ap()
    wqT_d = nc.dram_tensor("wqT", [HL, 128, NE, D], BF16,
                           kind="ExternalInput").ap()
    wkT_d = nc.dram_tensor("wkT", [128, NE, D], BF16,
                           kind="ExternalInput").ap()
    wvT_d = nc.dram_tensor("wvT", [128, NE, D], BF16,
                           kind="ExternalInput").ap()
    woT_d = nc.dram_tensor("woT", [128, HL, E], BF16,
                           kind="ExternalInput").ap()
    cosq_d = nc.dram_tensor("cosq", [D, S], BF16, kind="ExternalInput").ap()
    sinq_d = nc.dram_tensor("sinq", [D, S], BF16, kind="ExternalInput").ap()
    cosk_d = nc.dram_tensor("cosk", [D, S], BF16, kind="ExternalInput").ap()
    sink_d = nc.dram_tensor("sink", [D, S], BF16, kind="ExternalInput").ap()
    onesm_d = nc.dram_tensor("onesm", [128, 128], BF16,
                             kind="ExternalInput").ap()
    pswap_d = nc.dram_tensor("pswap", [128, 128], BF16,
                             kind="ExternalInput").ap()
    identb_d = nc.dram_tensor("identb", [128, 128], BF16,
                              kind="ExternalInput").ap()
    ident_d = nc.dram_tensor("ident", [128, 128], F32,
                             kind="ExternalInput").ap()
    if mode == "causal":
        cmaskT_d = nc.dram_tensor("cmaskT", [128, 128], BF16,
                                  kind="ExternalInput").ap()
    elif mode == "general":
        maskT_d = nc.dram_tensor("maskT", [S, S], BF16,
                                 kind="ExternalInput").ap()
    outp_d = nc.dram_tensor("outp", [NK, 128, E], BF16,
                            kind="ExternalOutput").ap()

    with tile.TileContext(nc) as tc:
      with nc.allow_low_precision(reason="bf16 attention kernel"), \
           tc.tile_pool(name="const", bufs=1) as cp, \
           tc.tile_pool(name="persist", bufs=1) as pp, \
           tc.tile_pool(name="hstp", bufs=5) as hst_pool, \
           tc.tile_pool(name="qrop", bufs=2) as qro_pool, \
           tc.tile_pool(name="aop", bufs=2) as ao_pool, \
           tc.tile_pool(name="xsp", bufs=2) as xs_pool, \
           tc.tile_pool(name="t12p", bufs=2) as t12_pool, \
           tc.tile_pool(name="vsp", bufs=1) as vs_pool, \
           tc.tile_pool(name="accp", bufs=1) as acc_pool, \
           tc.tile_pool(name="exp", bufs=5) as ex_pool, \
           tc.tile_pool(name="rcp", bufs=2) as rc_pool, \
           tc.tile_pool(name="ostp", bufs=2) as ost_pool, \
           tc.tile_pool(name="mtp", bufs=4) as mt_pool, \
           tc.tile_pool(name="ps", bufs=1, space="PSUM") as ps:

        # --- persistent SBUF ---
        wq_sb = pp.tile([128, HL, NE, D], BF16, name="wq_sb")
        wk_sb = pp.tile([128, NE, D], BF16, name="wk_sb")
        wv_sb = pp.tile([128, NE, D], BF16, name="wv_sb")
        wo_sb = pp.tile([128, HL, E], BF16, name="wo_sb")
        cq_sb = pp.tile([128, S], BF16, name="cq_sb")
        sq_sb = pp.tile([128, S], BF16, name="sq_sb")
        ck_sb = pp.tile([128, S], BF16, name="ck_sb")
        sk_sb = pp.tile([128, S], BF16, name="sk_sb")
        krope = pp.tile([128, S], BF16, name="krope")
        vnat = pp.tile([128, NK, D], BF16, name="vnat")
        ones_sb = cp.tile([128, 128], BF16, name="ones_sb")
        pswap = cp.tile([128, 128], BF16, name="pswap")
        identb = cp.tile([128, 128], BF16, name="identb")
        ident = cp.tile([128, 128], F32, name="ident")
        if mode == "causal":
            cmaskT = cp.tile([128, 128], BF16, name="cmaskT")

        hst = {}

        def issue_hst(g, b):
            t = hst_pool.tile([128, EB, TG], BF16, tag="hst",
                              name=f"hst{g}_{b}")
            nc.sync.dma_start(out=t, in_=hsT_d[g, b])
            hst[(g, b)] = t

        def hs_chunk(g, e):
            return hst[(g, e // EB)][:, e % EB, :]

        # --- startup DMAs on the sync hardware queue, ordered along the
        # k/v chain consumption order; first pieces are small so the first
        # matmul fires as early as possible. ---
        h0 = {}
        for b in range(NB):
            h0[b] = hst_pool.tile([128, EB, TG], BF16, tag="hst",
                                  name=f"hst0_{b}")
            hst[(0, b)] = h0[b]

        nc.sync.dma_start(out=wk_sb[:, 0:2, :], in_=wkT_d[:, 0:2, :])
        nc.sync.dma_start(out=h0[0][:, 0:2, :], in_=hsT_d[0, 0][:, 0:2, :])
        nc.sync.dma_start(out=wv_sb[:, 0:2, :], in_=wvT_d[:, 0:2, :])
        nc.sync.dma_start(out=wk_sb[:, 2:8, :], in_=wkT_d[:, 2:8, :])
        nc.sync.dma_start(out=h0[0][:, 2:8, :], in_=hsT_d[0, 0][:, 2:8, :])
        nc.sync.dma_start(out=wv_sb[:, 2:8, :], in_=wvT_d[:, 2:8, :])
        nc.sync.dma_start(out=wk_sb[:, 8:20, :], in_=wkT_d[:, 8:20, :])
        nc.sync.dma_start(out=h0[1], in_=hsT_d[0, 1])
        nc.sync.dma_start(out=wv_sb[:, 8:20, :], in_=wvT_d[:, 8:20, :])
        nc.sync.dma_start(out=wk_sb[:, 20:32, :], in_=wkT_d[:, 20:32, :])
        nc.sync.dma_start(out=h0[2], in_=hsT_d[0, 2])
        nc.sync.dma_start(out=wv_sb[:, 20:32, :], in_=wvT_d[:, 20:32, :])
        nc.sync.dma_start(out=h0[3], in_=hsT_d[0, 3])
        nc.sync.dma_start(out=ident, in_=ident_d)
        nc.sync.dma_start(out=pswap, in_=pswap_d)
        # rope tables stream in lazily: only group 0's slices at startup
        # (saves 1.5MB off the startup-critical DMA window); later slices
        # are prefetched inside attention(g) alongside the hst blocks
        nc.sync.dma_start(out=ck_sb[:, 0:TG], in_=cosk_d[:, 0:TG])
        nc.sync.dma_start(out=sk_sb[:, 0:TG], in_=sink_d[:, 0:TG])
        nc.sync.dma_start(out=wq_sb[:, 0], in_=wqT_d[0])
        nc.sync.dma_start(out=cq_sb[:, 0:TG], in_=cosq_d[:, 0:TG])
        nc.sync.dma_start(out=sq_sb[:, 0:TG], in_=sinq_d[:, 0:TG])
        nc.sync.dma_start(out=wq_sb[:, 1], in_=wqT_d[1])
        nc.sync.dma_start(out=identb, in_=identb_d)
        if mode == "causal":
            nc.sync.dma_start(out=cmaskT, in_=cmaskT_d)
        nc.sync.dma_start(out=wq_sb[:, 2], in_=wqT_d[2])
        nc.sync.dma_start(out=wq_sb[:, 3], in_=wqT_d[3])
        nc.sync.dma_start(out=ones_sb, in_=onesm_d)
        nc.sync.dma_start(out=wo_sb[:, :, :E // 2], in_=woT_d[:, :, :E // 2])
        nc.sync.dma_start(out=wo_sb[:, :, E // 2:], in_=woT_d[:, :, E // 2:])

        # --- helpers ---

        def rope_dve(x_ps, cos_sb, sin_sb, g, out_ap):
            """out = x*cos + rot_half(x)*sin; the half-rotation is two
            SBUF->SBUF partition-offset DMAs (DVE ops cannot cross partition
            offsets - the BIR verifier requires same start partitions - and
            a PE permutation matmul would cost 512 PE cycles per rope).
            Rope latency is off the critical path (the consuming scores run
            several microseconds later). Sign of the rotation is folded into
            sin_sb's first half; all operands stay bf16 SBUF so the DVE runs
            in 2x mode."""
            t0 = g * TG
            xs = xs_pool.tile([128, TG], BF16, tag="xs", name="xs")
            nc.scalar.copy(out=xs, in_=x_ps)
            sw = ps.tile([128, TG], F32, tag="DN", name="sw")
            nc.tensor.matmul(sw, pswap, xs, start=True, stop=True)
            t1 = t12_pool.tile([128, TG], BF16, tag="t1", name="t1")
            t2 = t12_pool.tile([128, TG], BF16, tag="t2", name="t2")
            nc.vector.tensor_mul(t1, xs, cos_sb[:, t0:t0 + TG])
            nc.vector.tensor_mul(t2, sw, sin_sb[:, t0:t0 + TG])
            nc.vector.tensor_add(out_ap, t1, t2)

        def kv_chains(g):
            """k and v chains interleaved chunk-by-chunk, then k-rope and
            the v transpose into vnat."""
            t0 = g * TG
            k_ps = ps.tile([128, TG], F32, tag="SP0", name=f"k_ps{g}")
            v_ps = ps.tile([128, TG], F32, tag="SP1", name=f"v_ps{g}")
            for e in range(NE):
                he = hs_chunk(g, e)
                nc.tensor.matmul(k_ps, wk_sb[:, e, :], he,
                                 start=(e == 0), stop=(e == NE - 1))
                nc.tensor.matmul(v_ps, wv_sb[:, e, :], he,
                                 start=(e == 0), stop=(e == NE - 1))
            rope_dve(k_ps, ck_sb, sk_sb, g, krope[:, t0:t0 + TG])
            vs = vs_pool.tile([128, TG], F32, tag="vs", name="vs")
            nc.scalar.copy(out=vs, in_=v_ps)
            for j in range(4):
                # alternate banks so transpose j+1 overlaps copy j
                tr = ps.tile([128, 128], F32, tag=("DN", "SP2")[j % 2],
                             name="tr")
                nc.tensor.transpose(tr, vs[:, 128 * j:128 * (j + 1)], ident)
                nc.vector.tensor_copy(vnat[:, 4 * g + j, :], tr)

        def qchain_units(g, f, tag, qro_t):
            """One emission callable per e-chunk of q-head f's chain; the
            last one also emits the rope."""
            holder = {}

            def mk(e):
                def emit():
                    if e == 0:
                        holder["ps"] = ps.tile([128, TG], F32, tag=tag,
                                               name=f"q_ps{g}_{f}")
                    nc.tensor.matmul(holder["ps"], wq_sb[:, f, e, :],
                                     hs_chunk(g, e),
                                     start=(e == 0), stop=(e == NE - 1))
                    if e == NE - 1:
                        rope_dve(holder["ps"], cq_sb, sq_sb, g,
                                 qro_t[:, f, :])
                return emit

            return [mk(e) for e in range(NE)]

        ao_tiles = {}
        qro_tiles = {}

        def oproj_pairs(gg, tis):
            """Yield one emission callable per o_proj matmul of group gg,
            token tiles `tis` only (braided as filler into attention(gg+1)).
            eg pairs use the OA/OB PSUM banks, accumulating over the 4 local
            heads; drains alternate scalar/DVE; half-row DMAs ride along."""
            ao_t = ao_tiles[gg]
            for ti in tis:
                ost = ost_pool.tile([128, E], BF16, tag="ost",
                                    name=f"ost{gg}_{ti}")
                state = {}
                for pair in range(4):
                    for h in range(HL):
                        for sub in range(2):
                            def emit(pair=pair, h=h, sub=sub, ti=ti,
                                     ost=ost, state=state):
                                eg = 2 * pair + sub
                                if h == 0:
                                    state[sub] = ps.tile(
                                        [128, TG], F32,
                                        tag="OA" if sub == 0 else "OB",
                                        name=f"op{gg}_{ti}_{eg}")
                                nc.tensor.matmul(
                                    state[sub],
                                    ao_t[:, h, 128 * ti:128 * (ti + 1)],
                                    wo_sb[:, h, TG * eg:TG * (eg + 1)],
                                    start=(h == 0), stop=(h == HL - 1))
                                if h == HL - 1 and sub == 0:
                                    nc.scalar.copy(
                                        out=ost[:, TG * eg:TG * (eg + 1)],
                                        in_=state[sub])
                                elif h == HL - 1:
                                    nc.vector.tensor_copy(
                                        ost[:, TG * eg:TG * (eg + 1)],
                                        state[sub])
                                    ro = 4 * gg + ti
                                    if pair == 1:
                                        nc.sync.dma_start(
                                            out=outp_d[ro][:, :E // 2],
                                            in_=ost[:, :E // 2])
                                    elif pair == 3:
                                        nc.sync.dma_start(
                                            out=outp_d[ro][:, E // 2:],
                                            in_=ost[:, E // 2:])
                            yield emit

        OTAGS = ["SP0", "SP1", "SP2", "AV0", "AV1", "DN", "OA", "OB"]

        def oproj_wide(gg, tis, last_group):
            """Baseline-style o_proj for token tiles `tis`: all 8 PSUM banks
            accumulate over the 4 local heads (ao stationary loaded once per
            (ti, h)), so drains never stall the PE. Runs only outside the
            braid (after attention, when the attention banks are free). The
            very last tile drains under the h3 sweep and DMAs out in
            quarters so the final DMA is small."""
            ao_t = ao_tiles[gg]
            for ti in tis:
                ost = ost_pool.tile([128, E], BF16, tag="ost",
                                    name=f"ost{gg}_{ti}")
                last = last_group and ti == 3
                ops = {}

                def drain_eg(eg, ost=ost, ti=ti, last=last):
                    dst = ost[:, TG * eg:TG * (eg + 1)]
                    if eg % 2:
                        nc.vector.tensor_copy(dst, ops[eg])
                    else:
                        nc.scalar.copy(out=dst, in_=ops[eg])
                    ro = 4 * gg + ti
                    if last:
                        # eighth-granular so the final transfer is tiny
                        nc.sync.dma_start(
                            out=outp_d[ro][:, TG * eg:TG * (eg + 1)],
                            in_=ost[:, TG * eg:TG * (eg + 1)])
                    else:
                        if eg == 3:
                            nc.sync.dma_start(out=outp_d[ro][:, :E // 2],
                                              in_=ost[:, :E // 2])
                        elif eg == 7:
                            nc.sync.dma_start(out=outp_d[ro][:, E // 2:],
                                              in_=ost[:, E // 2:])

                for h in range(HL):
                    for eg in range(E // TG):
                        if h == 0:
                            ops[eg] = ps.tile([128, TG], F32, tag=OTAGS[eg],
                                              name=f"opw{eg}")
                        nc.tensor.matmul(
                            ops[eg], ao_t[:, h, 128 * ti:128 * (ti + 1)],
                            wo_sb[:, h, TG * eg:TG * (eg + 1)],
                            start=(h == 0), stop=(h == HL - 1))
                        if last and h == HL - 1:
                            drain_eg(eg)
                if not last:
                    for eg in range(E // TG):
                        drain_eg(eg)

        def attention(g, fillers):
            """Flat pipelined (head, key-tile) loop; pulls one filler PE-op
            per iteration; issues hst(g+1) prefetches at milestones."""
            t0 = g * TG
            nk = 4 * g + 4 if mode == "causal" else NK
            qro_t = qro_tiles[g]
            ao_t = ao_tiles[g]
            av_ps = {}
            acc = {}
            queue = []
            srot = [0]

            def front(h, ki):
                c0 = 128 * (ki - 4 * g) if (mode == "causal" and ki >= 4 * g) \
                    else 0
                sp = ps.tile([128, TG], F32, tag=f"SP{srot[0] % 3}",
                             name="sp")
                srot[0] += 1
                diag = mode == "causal" and ki >= 4 * g
                nc.tensor.matmul(sp[:, c0:],
                                 krope[:, 128 * ki:128 * (ki + 1)],
                                 qro_t[:, h, c0:], start=True,
                                 stop=not diag, skip_group_check=True)
                if diag:
                    # += cmask via PE: cmask^T stationary x identity moving
                    nc.tensor.matmul(sp[:, c0:c0 + 128], cmaskT, identb,
                                     start=False, stop=True,
                                     skip_group_check=True)
                elif mode == "general":
                    mt = mt_pool.tile([128, TG], BF16, tag="mt", name="mt")
                    nc.sync.dma_start(
                        out=mt,
                        in_=maskT_d[128 * ki:128 * (ki + 1), t0:t0 + TG])
                    nc.vector.tensor_add(sp, sp, mt)
                ex = ex_pool.tile([128, TG], BF16, tag="ex", name="ex")
                nc.scalar.activation(out=ex[:, c0:], in_=sp[:, c0:], func=EXP)
                return (h, ki, c0, ex)

            def drain_one():
                h, ki, c0, ex = queue.pop(0)
                if ki == 0:
                    av_ps[h] = ps.tile([128, TG], F32, tag=f"AV{h % 2}",
                                       name=f"av{h}")
                    acc[h] = acc_pool.tile([128, TG], BF16, tag=f"acc{h}",
                                           name=f"acc{g}_{h}")
                last = (ki == nk - 1)
                nc.tensor.matmul(av_ps[h][:, c0:], vnat[:, ki, :],
                                 ex[:, c0:], start=(ki == 0), stop=last,
                                 skip_group_check=True)
                if ki == 0:
                    nc.vector.tensor_copy(acc[h], ex)
                else:
                    nc.vector.tensor_add(acc[h][:, c0:], acc[h][:, c0:],
                                         ex[:, c0:])
                if last:
                    den = ps.tile([128, TG], F32, tag="DN", name=f"den{h}")
                    nc.tensor.matmul(den, ones_sb, acc[h],
                                     start=True, stop=True)
                    rc = rc_pool.tile([128, TG], F32, tag="rc", name="rc")
                    nc.vector.reciprocal_approx_fast(out=rc, in_=den)
                    nc.vector.tensor_mul(ao_t[:, h, :], av_ps[h], rc)

            # hst(g+1) prefetch milestones. g=0 is special: the braided
            # q2/q3 chains still READ hst(0,*) buffers during early
            # iterations; each block's DMA must be emitted only after the
            # braided reads of the buffer it reuses (5-buffer rotation).
            miles = {1: 0, 6: 1, 7: 2, 8: 3} if g == 0 else \
                    {1: 0, 5: 1, 9: 2, 13: 3}
            nev = 0
            for h in range(HL):
                for ki in range(nk):
                    queue.append(front(h, ki))
                    nev += 1
                    if g < NG - 1 and nev in miles:
                        issue_hst(g + 1, miles[nev])
                    if g < NG - 1 and nev == 2:
                        t1g = (g + 1) * TG
                        for tb_sb, tb_d in ((ck_sb, cosk_d), (sk_sb, sink_d),
                                            (cq_sb, cosq_d), (sq_sb, sinq_d)):
                            nc.sync.dma_start(
                                out=tb_sb[:, t1g:t1g + TG],
                                in_=tb_d[:, t1g:t1g + TG])
                    for fl in fillers:
                        fl()
                        break
                    if len(queue) >= 3:
                        drain_one()
            while queue:
                drain_one()

        # ---- prologue: QKV(0); q2/q3 braid into attn(0) ----
        qro_tiles[0] = qro_pool.tile([128, HL, TG], BF16, tag="qro",
                                     name="qro0")
        kv_chains(0)
        for f in (0, 1):
            for u in qchain_units(0, f, ("OA", "OB")[f], qro_tiles[0]):
                u()

        def prologue_filler_gen():
            # 8 chain chunks per pull: q2 fully roped by iteration 3 (h=0),
            # q3 by iteration 7 (h=1) - both well before their heads' scores
            units = (qchain_units(0, 2, "OA", qro_tiles[0])
                     + qchain_units(0, 3, "OB", qro_tiles[0]))
            for i in range(0, len(units), 8):
                def emit8(i=i):
                    for u in units[i:i + 8]:
                        u()
                yield emit8

        # ---- supersteps ----
        # braided tis of oproj(g-1): just enough pair-mode units to cover
        # every attention(g) iteration (32 units per ti); the rest runs
        # wide (8-bank) right after attention, stall-free.
        BRAID_TIS = {1: (0,), 2: (0, 1), 3: (0, 1)}
        for g in range(NG):
            ao_tiles[g] = ao_pool.tile([128, HL, TG], BF16, tag="ao",
                                       name=f"ao{g}")
            if g == 0:
                fillers = prologue_filler_gen()
            else:
                fillers = oproj_pairs(g - 1, BRAID_TIS[g])
            attention(g, fillers)
            for fl in fillers:
                fl()
            if g >= 1:
                rest = tuple(t for t in range(4) if t not in BRAID_TIS[g])
                oproj_wide(g - 1, rest, last_group=False)
            if g < NG - 1:
                kv_chains(g + 1)
                qro_tiles[g + 1] = qro_pool.tile([128, HL, TG], BF16,
                                                 tag="qro", name=f"qro{g+1}")
                for f in range(HL):
                    for u in qchain_units(g + 1, f, ("OA", "OB")[f % 2],
                                          qro_tiles[g + 1]):
                        u()

        # ---- coda: o_proj of the last group, all wide ----
        oproj_wide(NG - 1, (0, 1, 2, 3), last_group=True)

    nc.compile()
    return nc


_CONSTS = None


def _consts():
    global _CONSTS
    if _CONSTS is None:
        kp = np.arange(128)[:, None]
        qc = np.arange(128)[None, :]
        cmask = np.where(qc >= kp, 0.0, NEG).astype(np.float32)
        cmaskT = np.ascontiguousarray(cmask.T).astype(BF)
        ones = np.ones((128, 128), dtype=BF)
        identb = np.eye(128, dtype=np.float32).astype(BF)
        ident = np.eye(128, dtype=np.float32)
        pswap = np.roll(np.eye(128, dtype=np.float32), 64, axis=0).astype(BF)
        _CONSTS = (cmaskT, ones, identb, ident, pswap)
    return _CONSTS


def _rope_tables(position_ids):
    pos = np.asarray(position_ids[0]).astype(np.float32)          # [S]
    inv_freq = (1.0 / (10000.0 ** (np.arange(0, D, 2, dtype=np.float32) / D)))
    freqs = pos[:, None] * inv_freq[None, :]                      # [S, 64]
    emb = np.concatenate([freqs, freqs], axis=1)                  # [S, 128]
    cosT = np.cos(emb).T.astype(np.float32).copy()                # [128, S]
    sinT = np.sin(emb).T.astype(np.float32)
    sinflipT = np.concatenate([-sinT[:64], sinT[64:]], axis=0)
    sc = np.float32(1.0 / np.sqrt(D))
    return ((cosT * sc).astype(BF), (sinflipT * sc).astype(BF),
            cosT.astype(BF), sinflipT.astype(BF))


def _pack_head(wT):
    """[E, dout] -> [128, NE, dout] with partition = within-chunk row."""
    dout = wT.shape[1]
    return np.ascontiguousarray(
        wT.reshape(NE, 128, dout).transpose(1, 0, 2)).astype(BF)


def kernel(hidden_states, position_ids, attention_mask, Wq, Wk, Wv, Wo):
    hidden_states = np.asarray(hidden_states)
    B = hidden_states.shape[0]
    assert hidden_states.shape == (B, S, E), hidden_states.shape
    assert B == 1

    mask = np.asarray(attention_mask, dtype=np.float32)[0, 0]
    if not mask.any():
        mode = "full"
    elif np.array_equal(mask, np.triu(np.full((S, S), NEG, dtype=np.float32), 1)):
        mode = "causal"
    else:
        mode = "general"

    if mode not in _PROGRAMS:
        _PROGRAMS[mode] = _build_program(mode)
    nc = _PROGRAMS[mode]

    hs = np.asarray(hidden_states[0], dtype=np.float32)
    # [E, S] -> packed blocks [NG, NB, 128, EB, TG]: E factored as
    # (NB, EB, 128) chunk-major (e = c*128 + p), S as (NG, TG); every
    # per-partition DMA line is then EB*TG*2 = 8KB contiguous.
    hsT = np.ascontiguousarray(
        hs.T.reshape(NB, EB, 128, NG, TG).transpose(3, 0, 2, 1, 4)).astype(BF)
    cosq, sinq, cosk, sink = _rope_tables(np.asarray(position_ids))
    Wq = np.asarray(Wq, dtype=np.float32)
    Wk = np.asarray(Wk, dtype=np.float32)
    Wv = np.asarray(Wv, dtype=np.float32)
    Wo = np.asarray(Wo, dtype=np.float32)
    cmaskT, ones, identb, ident, pswap = _consts()

    in_maps = []
    for c in range(NCORES):
        wq_c = Wq[512 * c:512 * (c + 1), :]            # [512, E] head-major
        wq_pack = np.stack([_pack_head(wq_c[128 * h:128 * (h + 1), :].T)
                            for h in range(HL)])       # [HL, 128, NE, D]
        wk_pack = _pack_head(Wk[128 * c:128 * (c + 1), :].T)
        wv_pack = _pack_head(Wv[128 * c:128 * (c + 1), :].T)
        # wo: [E, 512] slice -> [512, E] -> [HL, 128, E] -> [128, HL, E]
        wo_pack = np.ascontiguousarray(
            Wo[:, 512 * c:512 * (c + 1)].T.reshape(HL, 128, E)
            .transpose(1, 0, 2)).astype(BF)
        m = {
            "hsT": hsT,
            "wqT": wq_pack,
            "wkT": wk_pack,
            "wvT": wv_pack,
            "woT": wo_pack,
            "cosq": cosq, "sinq": sinq, "cosk": cosk, "sink": sink,
            "onesm": ones, "identb": identb, "ident": ident,
            "pswap": pswap,
        }
        if mode == "causal":
            m["cmaskT"] = cmaskT
        elif mode == "general":
            m["maskT"] = np.ascontiguousarray(mask.T).astype(BF)
        in_maps.append(m)

    res = run_bass_kernel_spmd(nc, in_maps, core_ids=list(range(NCORES)),
                               trace=TRACE[0])
    LAST_EXEC_NS[0] = res.exec_time_ns
    LAST_RES[0] = res

    acc = np.zeros((NK, 128, E), dtype=np.float32)
    for c in range(NCORES):
        acc += res.results[c]["outp"].astype(np.float32)
    out = acc.reshape(S, E)
    return out[None, :, :]


# revision 42
# speedup vs baseline: 1.0312x; 1.0027x over previous
"""Llama GQA attention layer (B=1, S=2048, E=4096, H=32, HKV=8, D=128) on 8
Trainium2 NeuronCores.

Sharding: tensor-parallel over heads. Core c owns Q heads 4c..4c+3 and KV head
c (KV groups stay intact), plus the matching Wo input-dim slice. Each core
computes a full [S, E] partial of the o_proj output; the host sums the 8
partials (the "all-reduce after o_proj").

All matmul operands are bf16 (PSUM accumulation fp32). The PE is the
bottleneck engine (~92% busy at 2.4 GHz), so everything that is not an
irreducible matmul is moved off it, and the PE stream is braided so it
never waits on the scalar engine:

  QKV: k and v chains interleaved chunk-by-chunk (DMA-paced at startup),
    q chains after. RoPE: half-rotation via a PE permutation matmul (sign
    folded into the sin table, 1/sqrt(D) folded into q's cos/sin tables;
    a DVE rotate is illegal - same-start-partition rule - and an SBUF DMA
    rotate loses more to sync-queue contention than the 512 PE cycles),
    then 3 DVE ops. v is transposed to [tok, d] via PE transpose, banks
    alternating so copies overlap transposes.
  attention: flat software-pipelined loop over (head, key-tile); scoresT[k,q]
    -> exp on ScalarE -> av accumulates in PSUM. The softmax denominator is
    NOT a per-tile PE matmul (that would cost 69k PE cycles): exp tiles are
    accumulated elementwise in bf16 on the DVE (den error ~0.1% after the
    final exact 128-way matmul sum) and a single ones-matmul per (head,
    group) broadcasts the denominator into every PSUM row. The causal
    diagonal mask is added by the PE itself: a 128-col matmul (cmask^T
    stationary x identity moving) appended to the score accumulation.
    Softmax has no max subtraction (scores ~ N(0,1), exp cannot overflow).
    Epilogue: reciprocal_approx_fast (DVE) + one DVE mul -> ao.
  braid: after den-removal the attention inner loop is scalar-exp-paced
    (~530ns/iter vs ~390ns PE), so o_proj matmuls of the PREVIOUS group are
    braided into the attention loop one per iteration (eg-pair structure on
    2 PSUM banks); group 0 braids the tail q-chains of its own QKV instead
    (8 chunks/iter so every qro head is roped before its scores are emitted
    - emission-order deadlock otherwise). QKV(g+1) runs after attention(g).
  o_proj outside the braid runs baseline-wide: all 8 PSUM banks accumulate
    over the 4 local heads, so drains (alternating scalar/DVE) never stall
    the PE; per-token-tile output DMAed bf16, the last tile in quarters so
    the final DMA is small. Host sums the 8 partial [S, E] outputs.

PSUM banks: SP0-2 (score rotation / k,v chains / vtr), AV0-1 (av by head
parity), DN (den broadcast / rope swap / vtr), OA-OB (o_proj pairs /
q chains) = 8; wide o_proj uses all 8 after attention retires.

DMA: all host-side arrays are packed so every descriptor moves long
contiguous per-partition lines (hsT blocks [128, 8*512], weights
[128, NE*D]); first pieces are small so the first matmul fires early;
hst(g+1) block prefetches are staggered inside attention(g) at milestones
that respect the 5-buffer rotation against the braided q-chain reads.
"""

import sys
import types

if "/opt/trn_rl_repo" not in sys.path:
    sys.path.insert(0, "/opt/trn_rl_repo")

import numpy as np
import ml_dtypes

import concourse.bass as bass
import concourse.tile as tile
from concourse import bacc, mybir
from concourse.bass_utils import run_bass_kernel_spmd

F32 = mybir.dt.float32
BF16 = mybir.dt.bfloat16
EXP = mybir.ActivationFunctionType.Exp
BF = ml_dtypes.bfloat16

S = 2048
E = 4096
H = 32
HKV = 8
D = 128
NCORES = 8
HL = H // NCORES          # 4 local q heads per core
TG = 512                  # token group (moving-dim tile)
NG = S // TG              # 4 token groups
NE = E // 128             # 32 contraction chunks
NK = S // 128             # 16 key tiles
EB = 8                    # e-chunks per hsT DMA block
NB = NE // EB             # 4 blocks per group
NEG = -1e9

TRACE = [False]
LAST_EXEC_NS = [None]
LAST_RES = [None]

_PROGRAMS = {}


def _install_ntff_hook():
    if "antenv.axon_hooks" in sys.modules:
        return
    mod = types.ModuleType("antenv.axon_hooks")
    hook = [None]
    mod.set_axon_ntff_profile_hook = lambda h: hook.__setitem__(0, h)
    mod.get_axon_ntff_profile_hook = lambda: hook[0]
    sys.modules["antenv.axon_hooks"] = mod
    try:
        from trn_agent_boot.trn_boot import _ntff_profile_via_ctypes

        mod.set_axon_ntff_profile_hook(
            _ntff_profile_via_ctypes("/opt/axon/libaxon_pjrt.so"))
    except Exception:
        pass


def set_trace(on=True):
    if on:
        _install_ntff_hook()
    TRACE[0] = on


def _build_program(mode):
    """mode: 'causal' (skip above-diagonal key tiles, live-column diagonal),
    'full' (no mask), 'general' (additive mask streamed from DRAM).
    """
    nc = bacc.Bacc(trn_type="TRN2", target_bir_lowering=False, debug=False)

    # packed DRAM layouts: per-partition lines fully contiguous
    hsT_d = nc.dram_tensor("hsT", [NG, NB, 128, EB, TG], BF16,
                           kind="ExternalInput").ap()
    wqT_d = nc.dram_tensor("wqT", [HL, 128, NE, D], BF16,
                           kind="ExternalInput").ap()
    wkT_d = nc.dram_tensor("wkT", [128, NE, D], BF16,
                           kind="ExternalInput").ap()
    wvT_d = nc.dram_tensor("wvT", [128, NE, D], BF16,
                           kind="ExternalInput").ap()
    woT_d = nc.dram_tensor("woT", [128, HL, E], BF16,
                           kind="ExternalInput").ap()
    cosq_d = nc.dram_tensor("cosq", [D, S], BF16, kind="ExternalInput").ap()
    sinq_d = nc.dram_tensor("sinq", [D, S], BF16, kind="ExternalInput").ap()
    cosk_d = nc.dram_tensor("cosk", [D, S], BF16, kind="ExternalInput").ap()
    sink_d = nc.dram_tensor("sink", [D, S], BF16, kind="ExternalInput").ap()
    onesm_d = nc.dram_tensor("onesm", [128, 128], BF16,
                             kind="ExternalInput").ap()
    pswap_d = nc.dram_tensor("pswap", [128, 128], BF16,
                             kind="ExternalInput").ap()
    identb_d = nc.dram_tensor("identb", [128, 128], BF16,
                              kind="ExternalInput").ap()
    ident_d = nc.dram_tensor("ident", [128, 128], F32,
                             kind="ExternalInput").ap()
    if mode == "causal":
        cmaskT_d = nc.dram_tensor("cmaskT", [128, 128], BF16,
                                  kind="ExternalInput").ap()
    elif mode == "general":
        maskT_d = nc.dram_tensor("maskT", [S, S], BF16,
                                 kind="ExternalInput").ap()
    outp_d = nc.dram_tensor("outp", [NK, 128, E], BF16,
                            kind="ExternalOutput").ap()

    with tile.TileContext(nc) as tc:
      with nc.allow_low_precision(reason="bf16 attention kernel"), \
           tc.tile_pool(name="const", bufs=1) as cp, \
           tc.tile_pool(name="persist", bufs=1) as pp, \
           tc.tile_pool(name="hstp", bufs=5) as hst_pool, \
           tc.tile_pool(name="qrop", bufs=2) as qro_pool, \
           tc.tile_pool(name="aop", bufs=2) as ao_pool, \
           tc.tile_pool(name="xsp", bufs=2) as xs_pool, \
           tc.tile_pool(name="t12p", bufs=2) as t12_pool, \
           tc.tile_pool(name="vsp", bufs=1) as vs_pool, \
           tc.tile_pool(name="accp", bufs=1) as acc_pool, \
           tc.tile_pool(name="exp", bufs=5) as ex_pool, \
           tc.tile_pool(name="rcp", bufs=2) as rc_pool, \
           tc.tile_pool(name="ostp", bufs=2) as ost_pool, \
           tc.tile_pool(name="mtp", bufs=4) as mt_pool, \
           tc.tile_pool(name="ps", bufs=1, space="PSUM") as ps:

        # --- persistent SBUF ---
        wq_sb = pp.tile([128, HL, NE, D], BF16, name="wq_sb")
        wk_sb = pp.tile([128, NE, D], BF16, name="wk_sb")
        wv_sb = pp.tile([128, NE, D], BF16, name="wv_sb")
        wo_sb = pp.tile([128, HL, E], BF16, name="wo_sb")
        cq_sb = pp.tile([128, S], BF16, name="cq_sb")
        sq_sb = pp.tile([128, S], BF16, name="sq_sb")
        ck_sb = pp.tile([128, S], BF16, name="ck_sb")
        sk_sb = pp.tile([128, S], BF16, name="sk_sb")
        krope = pp.tile([128, S], BF16, name="krope")
        vnat = pp.tile([128, NK, D], BF16, name="vnat")
        ones_sb = cp.tile([128, 128], BF16, name="ones_sb")
        pswap = cp.tile([128, 128], BF16, name="pswap")
        identb = cp.tile([128, 128], BF16, name="identb")
        ident = cp.tile([128, 128], F32, name="ident")
        if mode == "causal":
            cmaskT = cp.tile([128, 128], BF16, name="cmaskT")

        hst = {}

        def issue_hst(g, b):
            t = hst_pool.tile([128, EB, TG], BF16, tag="hst",
                              name=f"hst{g}_{b}")
            nc.sync.dma_start(out=t, in_=hsT_d[g, b])
            hst[(g, b)] = t

        def hs_chunk(g, e):
            return hst[(g, e // EB)][:, e % EB, :]

        # --- startup DMAs on the sync hardware queue, ordered along the
        # k/v chain consumption order; first pieces are small so the first
        # matmul fires as early as possible. ---
        h0 = {}
        for b in range(NB):
            h0[b] = hst_pool.tile([128, EB, TG], BF16, tag="hst",
                                  name=f"hst0_{b}")
            hst[(0, b)] = h0[b]

        # wq0 ranges are interleaved with the k/v streams: the prologue
        # runs k, v AND q0 chains as a triple consumer (3 matmuls = 639ns
        # of PE per e-chunk >= the ~600ns DMA arrival rate), so the PE
        # never starves while the startup stream lands.
        nc.sync.dma_start(out=wk_sb[:, 0:2, :], in_=wkT_d[:, 0:2, :])
        nc.sync.dma_start(out=h0[0][:, 0:2, :], in_=hsT_d[0, 0][:, 0:2, :])
        nc.sync.dma_start(out=wv_sb[:, 0:2, :], in_=wvT_d[:, 0:2, :])
        nc.sync.dma_start(out=wq_sb[:, 0, 0:2, :], in_=wqT_d[0][:, 0:2, :])
        nc.sync.dma_start(out=wk_sb[:, 2:8, :], in_=wkT_d[:, 2:8, :])
        nc.sync.dma_start(out=h0[0][:, 2:8, :], in_=hsT_d[0, 0][:, 2:8, :])
        nc.sync.dma_start(out=wv_sb[:, 2:8, :], in_=wvT_d[:, 2:8, :])
        nc.sync.dma_start(out=wq_sb[:, 0, 2:8, :], in_=wqT_d[0][:, 2:8, :])
        nc.sync.dma_start(out=wk_sb[:, 8:20, :], in_=wkT_d[:, 8:20, :])
        nc.sync.dma_start(out=h0[1], in_=hsT_d[0, 1])
        nc.sync.dma_start(out=wv_sb[:, 8:20, :], in_=wvT_d[:, 8:20, :])
        nc.sync.dma_start(out=wq_sb[:, 0, 8:20, :], in_=wqT_d[0][:, 8:20, :])
        nc.sync.dma_start(out=wk_sb[:, 20:32, :], in_=wkT_d[:, 20:32, :])
        nc.sync.dma_start(out=h0[2], in_=hsT_d[0, 2])
        nc.sync.dma_start(out=wv_sb[:, 20:32, :], in_=wvT_d[:, 20:32, :])
        nc.sync.dma_start(out=wq_sb[:, 0, 20:32, :],
                          in_=wqT_d[0][:, 20:32, :])
        nc.sync.dma_start(out=h0[3], in_=hsT_d[0, 3])
        nc.sync.dma_start(out=ident, in_=ident_d)
        nc.sync.dma_start(out=pswap, in_=pswap_d)
        # rope tables stream in lazily: only group 0's slices at startup
        # (saves 1.5MB off the startup-critical DMA window); later slices
        # are prefetched inside attention(g) alongside the hst blocks
        nc.sync.dma_start(out=ck_sb[:, 0:TG], in_=cosk_d[:, 0:TG])
        nc.sync.dma_start(out=sk_sb[:, 0:TG], in_=sink_d[:, 0:TG])
        nc.sync.dma_start(out=cq_sb[:, 0:TG], in_=cosq_d[:, 0:TG])
        nc.sync.dma_start(out=sq_sb[:, 0:TG], in_=sinq_d[:, 0:TG])
        nc.sync.dma_start(out=wq_sb[:, 1], in_=wqT_d[1])
        nc.sync.dma_start(out=identb, in_=identb_d)
        if mode == "causal":
            nc.sync.dma_start(out=cmaskT, in_=cmaskT_d)
        nc.sync.dma_start(out=wq_sb[:, 2], in_=wqT_d[2])
        nc.sync.dma_start(out=wq_sb[:, 3], in_=wqT_d[3])
        nc.sync.dma_start(out=ones_sb, in_=onesm_d)
        nc.sync.dma_start(out=wo_sb[:, :, :E // 2], in_=woT_d[:, :, :E // 2])
        nc.sync.dma_start(out=wo_sb[:, :, E // 2:], in_=woT_d[:, :, E // 2:])

        # --- helpers ---

        def rope_dve(x_ps, cos_sb, sin_sb, g, out_ap):
            """out = x*cos + rot_half(x)*sin; the half-rotation is two
            SBUF->SBUF partition-offset DMAs (DVE ops cannot cross partition
            offsets - the BIR verifier requires same start partitions - and
            a PE permutation matmul would cost 512 PE cycles per rope).
            Rope latency is off the critical path (the consuming scores run
            several microseconds later). Sign of the rotation is folded into
            sin_sb's first half; all operands stay bf16 SBUF so the DVE runs
            in 2x mode."""
            t0 = g * TG
            xs = xs_pool.tile([128, TG], BF16, tag="xs", name="xs")
            nc.scalar.copy(out=xs, in_=x_ps)
            sw = ps.tile([128, TG], F32, tag="DN", name="sw")
            nc.tensor.matmul(sw, pswap, xs, start=True, stop=True)
            t1 = t12_pool.tile([128, TG], BF16, tag="t1", name="t1")
            t2 = t12_pool.tile([128, TG], BF16, tag="t2", name="t2")
            nc.vector.tensor_mul(t1, xs, cos_sb[:, t0:t0 + TG])
            nc.vector.tensor_mul(t2, sw, sin_sb[:, t0:t0 + TG])
            nc.vector.tensor_add(out_ap, t1, t2)

        def kv_chains(g):
            """k and v chains interleaved chunk-by-chunk, then k-rope and
            the v transpose into vnat."""
            t0 = g * TG
            k_ps = ps.tile([128, TG], F32, tag="SP0", name=f"k_ps{g}")
            v_ps = ps.tile([128, TG], F32, tag="SP1", name=f"v_ps{g}")
            for e in range(NE):
                he = hs_chunk(g, e)
                nc.tensor.matmul(k_ps, wk_sb[:, e, :], he,
                                 start=(e == 0), stop=(e == NE - 1))
                nc.tensor.matmul(v_ps, wv_sb[:, e, :], he,
                                 start=(e == 0), stop=(e == NE - 1))
            rope_dve(k_ps, ck_sb, sk_sb, g, krope[:, t0:t0 + TG])
            vs = vs_pool.tile([128, TG], F32, tag="vs", name="vs")
            nc.scalar.copy(out=vs, in_=v_ps)
            for j in range(4):
                # alternate banks so transpose j+1 overlaps copy j
                tr = ps.tile([128, 128], F32, tag=("DN", "SP2")[j % 2],
                             name="tr")
                nc.tensor.transpose(tr, vs[:, 128 * j:128 * (j + 1)], ident)
                nc.vector.tensor_copy(vnat[:, 4 * g + j, :], tr)

        def qchain_units(g, f, tag, qro_t):
            """One emission callable per e-chunk of q-head f's chain; the
            last one also emits the rope."""
            holder = {}

            def mk(e):
                def emit():
                    if e == 0:
                        holder["ps"] = ps.tile([128, TG], F32, tag=tag,
                                               name=f"q_ps{g}_{f}")
                    nc.tensor.matmul(holder["ps"], wq_sb[:, f, e, :],
                                     hs_chunk(g, e),
                                     start=(e == 0), stop=(e == NE - 1))
                    if e == NE - 1:
                        rope_dve(holder["ps"], cq_sb, sq_sb, g,
                                 qro_t[:, f, :])
                return emit

            return [mk(e) for e in range(NE)]

        ao_tiles = {}
        qro_tiles = {}

        def oproj_pairs(gg, tis):
            """Yield one emission callable per o_proj matmul of group gg,
            token tiles `tis` only (braided as filler into attention(gg+1)).
            eg pairs use the OA/OB PSUM banks, accumulating over the 4 local
            heads; drains alternate scalar/DVE; half-row DMAs ride along."""
            ao_t = ao_tiles[gg]
            for ti in tis:
                ost = ost_pool.tile([128, E], BF16, tag="ost",
                                    name=f"ost{gg}_{ti}")
                state = {}
                for pair in range(4):
                    for h in range(HL):
                        for sub in range(2):
                            def emit(pair=pair, h=h, sub=sub, ti=ti,
                                     ost=ost, state=state):
                                eg = 2 * pair + sub
                                if h == 0:
                                    state[sub] = ps.tile(
                                        [128, TG], F32,
                                        tag="OA" if sub == 0 else "OB",
                                        name=f"op{gg}_{ti}_{eg}")
                                nc.tensor.matmul(
                                    state[sub],
                                    ao_t[:, h, 128 * ti:128 * (ti + 1)],
                                    wo_sb[:, h, TG * eg:TG * (eg + 1)],
                                    start=(h == 0), stop=(h == HL - 1))
                                if h == HL - 1 and sub == 0:
                                    nc.scalar.copy(
                                        out=ost[:, TG * eg:TG * (eg + 1)],
                                        in_=state[sub])
                                elif h == HL - 1:
                                    nc.vector.tensor_copy(
                                        ost[:, TG * eg:TG * (eg + 1)],
                                        state[sub])
                                    ro = 4 * gg + ti
                                    if pair == 1:
                                        nc.sync.dma_start(
                                            out=outp_d[ro][:, :E // 2],
                                            in_=ost[:, :E // 2])
                                    elif pair == 3:
                                        nc.sync.dma_start(
                                            out=outp_d[ro][:, E // 2:],
                                            in_=ost[:, E // 2:])
                            yield emit

        OTAGS = ["SP0", "SP1", "SP2", "AV0", "AV1", "DN", "OA", "OB"]

        def oproj_wide(gg, tis, last_group):
            """Baseline-style o_proj for token tiles `tis`: all 8 PSUM banks
            accumulate over the 4 local heads (ao stationary loaded once per
            (ti, h)), so drains never stall the PE. Runs only outside the
            braid (after attention, when the attention banks are free). The
            very last tile drains under the h3 sweep and DMAs out in
            quarters so the final DMA is small."""
            ao_t = ao_tiles[gg]
            for ti in tis:
                ost = ost_pool.tile([128, E], BF16, tag="ost",
                                    name=f"ost{gg}_{ti}")
                last = last_group and ti == 3
                ops = {}

                def drain_eg(eg, ost=ost, ti=ti, last=last):
                    dst = ost[:, TG * eg:TG * (eg + 1)]
                    if eg % 2:
                        nc.vector.tensor_copy(dst, ops[eg])
                    else:
                        nc.scalar.copy(out=dst, in_=ops[eg])
                    ro = 4 * gg + ti
                    if last:
                        # eighth-granular so the final transfer is tiny
                        nc.sync.dma_start(
                            out=outp_d[ro][:, TG * eg:TG * (eg + 1)],
                            in_=ost[:, TG * eg:TG * (eg + 1)])
                    else:
                        if eg == 3:
                            nc.sync.dma_start(out=outp_d[ro][:, :E // 2],
                                              in_=ost[:, :E // 2])
                        elif eg == 7:
                            nc.sync.dma_start(out=outp_d[ro][:, E // 2:],
                                              in_=ost[:, E // 2:])

                for h in range(HL):
                    for eg in range(E // TG):
                        if h == 0:
                            ops[eg] = ps.tile([128, TG], F32, tag=OTAGS[eg],
                                              name=f"opw{eg}")
                        nc.tensor.matmul(
                            ops[eg], ao_t[:, h, 128 * ti:128 * (ti + 1)],
                            wo_sb[:, h, TG * eg:TG * (eg + 1)],
                            start=(h == 0), stop=(h == HL - 1))
                        if last and h == HL - 1:
                            drain_eg(eg)
                if not last:
                    for eg in range(E // TG):
                        drain_eg(eg)

        def attention(g, fillers):
            """Flat pipelined (head, key-tile) loop; pulls one filler PE-op
            per iteration; issues hst(g+1) prefetches at milestones."""
            t0 = g * TG
            nk = 4 * g + 4 if mode == "causal" else NK
            qro_t = qro_tiles[g]
            ao_t = ao_tiles[g]
            av_ps = {}
            acc = {}
            queue = []
            srot = [0]

            def front(h, ki):
                c0 = 128 * (ki - 4 * g) if (mode == "causal" and ki >= 4 * g) \
                    else 0
                sp = ps.tile([128, TG], F32, tag=f"SP{srot[0] % 3}",
                             name="sp")
                srot[0] += 1
                diag = mode == "causal" and ki >= 4 * g
                nc.tensor.matmul(sp[:, c0:],
                                 krope[:, 128 * ki:128 * (ki + 1)],
                                 qro_t[:, h, c0:], start=True,
                                 stop=not diag, skip_group_check=True)
                if diag:
                    # += cmask via PE: cmask^T stationary x identity moving
                    nc.tensor.matmul(sp[:, c0:c0 + 128], cmaskT, identb,
                                     start=False, stop=True,
                                     skip_group_check=True)
                elif mode == "general":
                    mt = mt_pool.tile([128, TG], BF16, tag="mt", name="mt")
                    nc.sync.dma_start(
                        out=mt,
                        in_=maskT_d[128 * ki:128 * (ki + 1), t0:t0 + TG])
                    nc.vector.tensor_add(sp, sp, mt)
                ex = ex_pool.tile([128, TG], BF16, tag="ex", name="ex")
                nc.scalar.activation(out=ex[:, c0:], in_=sp[:, c0:], func=EXP)
                return (h, ki, c0, ex)

            def drain_one():
                h, ki, c0, ex = queue.pop(0)
                if ki == 0:
                    av_ps[h] = ps.tile([128, TG], F32, tag=f"AV{h % 2}",
                                       name=f"av{h}")
                    acc[h] = acc_pool.tile([128, TG], BF16, tag=f"acc{h}",
                                           name=f"acc{g}_{h}")
                last = (ki == nk - 1)
                nc.tensor.matmul(av_ps[h][:, c0:], vnat[:, ki, :],
                                 ex[:, c0:], start=(ki == 0), stop=last,
                                 skip_group_check=True)
                if ki == 0:
                    nc.vector.tensor_copy(acc[h], ex)
                else:
                    nc.vector.tensor_add(acc[h][:, c0:], acc[h][:, c0:],
                                         ex[:, c0:])
                if last:
                    den = ps.tile([128, TG], F32, tag="DN", name=f"den{h}")
                    nc.tensor.matmul(den, ones_sb, acc[h],
                                     start=True, stop=True)
                    rc = rc_pool.tile([128, TG], F32, tag="rc", name="rc")
                    nc.vector.reciprocal_approx_fast(out=rc, in_=den)
                    nc.vector.tensor_mul(ao_t[:, h, :], av_ps[h], rc)

            # hst(g+1) prefetch milestones. g=0 is special: the braided
            # q2/q3 chains still READ hst(0,*) buffers during early
            # iterations; each block's DMA must be emitted only after the
            # braided reads of the buffer it reuses (5-buffer rotation).
            miles = {1: 0, 6: 1, 7: 2, 8: 3} if g == 0 else \
                    {1: 0, 5: 1, 9: 2, 13: 3}
            nev = 0
            for h in range(HL):
                for ki in range(nk):
                    queue.append(front(h, ki))
                    nev += 1
                    if g < NG - 1 and nev in miles:
                        issue_hst(g + 1, miles[nev])
                    if g < NG - 1 and nev == 2:
                        t1g = (g + 1) * TG
                        for tb_sb, tb_d in ((ck_sb, cosk_d), (sk_sb, sink_d),
                                            (cq_sb, cosq_d), (sq_sb, sinq_d)):
                            nc.sync.dma_start(
                                out=tb_sb[:, t1g:t1g + TG],
                                in_=tb_d[:, t1g:t1g + TG])
                    for fl in fillers:
                        fl()
                        break
                    if len(queue) >= 3:
                        drain_one()
            while queue:
                drain_one()

        # ---- prologue: QKV(0) with k/v/q0 as a TRIPLE interleaved chain
        # (3 matmuls of PE work per arriving e-chunk - the startup DMA
        # stream can't starve the PE); q1 after; q2/q3 braid into attn(0).
        qro_tiles[0] = qro_pool.tile([128, HL, TG], BF16, tag="qro",
                                     name="qro0")
        k_ps = ps.tile([128, TG], F32, tag="SP0", name="k_ps0")
        v_ps = ps.tile([128, TG], F32, tag="SP1", name="v_ps0")
        q_ps = ps.tile([128, TG], F32, tag="OA", name="q_ps0_0")
        for e in range(NE):
            he = hs_chunk(0, e)
            nc.tensor.matmul(k_ps, wk_sb[:, e, :], he,
                             start=(e == 0), stop=(e == NE - 1))
            nc.tensor.matmul(v_ps, wv_sb[:, e, :], he,
                             start=(e == 0), stop=(e == NE - 1))
            nc.tensor.matmul(q_ps, wq_sb[:, 0, e, :], he,
                             start=(e == 0), stop=(e == NE - 1))
        rope_dve(k_ps, ck_sb, sk_sb, 0, krope[:, 0:TG])
        rope_dve(q_ps, cq_sb, sq_sb, 0, qro_tiles[0][:, 0, :])
        vs0 = vs_pool.tile([128, TG], F32, tag="vs", name="vs0")
        nc.scalar.copy(out=vs0, in_=v_ps)
        for j in range(4):
            tr = ps.tile([128, 128], F32, tag=("DN", "SP2")[j % 2],
                         name="tr")
            nc.tensor.transpose(tr, vs0[:, 128 * j:128 * (j + 1)], ident)
            nc.vector.tensor_copy(vnat[:, j, :], tr)
        for u in qchain_units(0, 1, "OB", qro_tiles[0]):
            u()

        def prologue_filler_gen():
            # 8 chain chunks per pull: q2 fully roped by iteration 3 (h=0),
            # q3 by iteration 7 (h=1) - both well before their heads' scores
            units = (qchain_units(0, 2, "OA", qro_tiles[0])
                     + qchain_units(0, 3, "OB", qro_tiles[0]))
            for i in range(0, len(units), 8):
                def emit8(i=i):
                    for u in units[i:i + 8]:
                        u()
                yield emit8

        # ---- supersteps ----
        # braided tis of oproj(g-1): just enough pair-mode units to cover
        # every attention(g) iteration (32 units per ti); the rest runs
        # wide (8-bank) right after attention, stall-free.
        BRAID_TIS = {1: (0,), 2: (0, 1), 3: (0, 1)}
        for g in range(NG):
            ao_tiles[g] = ao_pool.tile([128, HL, TG], BF16, tag="ao",
                                       name=f"ao{g}")
            if g == 0:
                fillers = prologue_filler_gen()
            else:
                fillers = oproj_pairs(g - 1, BRAID_TIS[g])
            attention(g, fillers)
            for fl in fillers:
                fl()
            if g >= 1:
                rest = tuple(t for t in range(4) if t not in BRAID_TIS[g])
                oproj_wide(g - 1, rest, last_group=False)
            if g < NG - 1:
                kv_chains(g + 1)
                qro_tiles[g + 1] = qro_pool.tile([128, HL, TG], BF16,
                                                 tag="qro", name=f"qro{g+1}")
                for f in range(HL):
                    for u in qchain_units(g + 1, f, ("OA", "OB")[f % 2],
                                          qro_tiles[g + 1]):
                        u()

        # ---- coda: o_proj of the last group, all wide ----
        oproj_wide(NG - 1, (0, 1, 2, 3), last_group=True)

    nc.compile()
    return nc


_CONSTS = None


def _consts():
    global _CONSTS
    if _CONSTS is None:
        kp = np.arange(128)[:, None]
        qc = np.arange(128)[None, :]
        cmask = np.where(qc >= kp, 0.0, NEG).astype(np.float32)
        cmaskT = np.ascontiguousarray(cmask.T).astype(BF)
        ones = np.ones((128, 128), dtype=BF)
        identb = np.eye(128, dtype=np.float32).astype(BF)
        ident = np.eye(128, dtype=np.float32)
        pswap = np.roll(np.eye(128, dtype=np.float32), 64, axis=0).astype(BF)
        _CONSTS = (cmaskT, ones, identb, ident, pswap)
    return _CONSTS


def _rope_tables(position_ids):
    pos = np.asarray(position_ids[0]).astype(np.float32)          # [S]
    inv_freq = (1.0 / (10000.0 ** (np.arange(0, D, 2, dtype=np.float32) / D)))
    freqs = pos[:, None] * inv_freq[None, :]                      # [S, 64]
    emb = np.concatenate([freqs, freqs], axis=1)                  # [S, 128]
    cosT = np.cos(emb).T.astype(np.float32).copy()                # [128, S]
    sinT = np.sin(emb).T.astype(np.float32)
    sinflipT = np.concatenate([-sinT[:64], sinT[64:]], axis=0)
    sc = np.float32(1.0 / np.sqrt(D))
    return ((cosT * sc).astype(BF), (sinflipT * sc).astype(BF),
            cosT.astype(BF), sinflipT.astype(BF))


def _pack_head(wT):
    """[E, dout] -> [128, NE, dout] with partition = within-chunk row."""
    dout = wT.shape[1]
    return np.ascontiguousarray(
        wT.reshape(NE, 128, dout).transpose(1, 0, 2)).astype(BF)


def kernel(hidden_states, position_ids, attention_mask, Wq, Wk, Wv, Wo):
    hidden_states = np.asarray(hidden_states)
    B = hidden_states.shape[0]
    assert hidden_states.shape == (B, S, E), hidden_states.shape
    assert B == 1

    mask = np.asarray(attention_mask, dtype=np.float32)[0, 0]
    if not mask.any():
        mode = "full"
    elif np.array_equal(mask, np.triu(np.full((S, S), NEG, dtype=np.float32), 1)):
        mode = "causal"
    else:
        mode = "general"

    if mode not in _PROGRAMS:
        _PROGRAMS[mode] = _build_program(mode)
    nc = _PROGRAMS[mode]

    hs = np.asarray(hidden_states[0], dtype=np.float32)
    # [E, S] -> packed blocks [NG, NB, 128, EB, TG]: E factored as
    # (NB, EB, 128) chunk-major (e = c*128 + p), S as (NG, TG); every
    # per-partition DMA line is then EB*TG*2 = 8KB contiguous.
    hsT = np.ascontiguousarray(
        hs.T.reshape(NB, EB, 128, NG, TG).transpose(3, 0, 2, 1, 4)).astype(BF)
    cosq, sinq, cosk, sink = _rope_tables(np.asarray(position_ids))
    Wq = np.asarray(Wq, dtype=np.float32)
    Wk = np.asarray(Wk, dtype=np.float32)
    Wv = np.asarray(Wv, dtype=np.float32)
    Wo = np.asarray(Wo, dtype=np.float32)
    cmaskT, ones, identb, ident, pswap = _consts()

    in_maps = []
    for c in range(NCORES):
        wq_c = Wq[512 * c:512 * (c + 1), :]            # [512, E] head-major
        wq_pack = np.stack([_pack_head(wq_c[128 * h:128 * (h + 1), :].T)
                            for h in range(HL)])       # [HL, 128, NE, D]
        wk_pack = _pack_head(Wk[128 * c:128 * (c + 1), :].T)
        wv_pack = _pack_head(Wv[128 * c:128 * (c + 1), :].T)
        # wo: [E, 512] slice -> [512, E] -> [HL, 128, E] -> [128, HL, E]
        wo_pack = np.ascontiguousarray(
            Wo[:, 512 * c:512 * (c + 1)].T.reshape(HL, 128, E)
            .transpose(1, 0, 2)).astype(BF)
        m = {
            "hsT": hsT,
            "wqT": wq_pack,
            "wkT": wk_pack,
            "wvT": wv_pack,
            "woT": wo_pack,
            "cosq": cosq, "sinq": sinq, "cosk": cosk, "sink": sink,
            "onesm": ones, "identb": identb, "ident": ident,
            "pswap": pswap,
        }
        if mode == "causal":
            m["cmaskT"] = cmaskT
        elif mode == "general":
            m["maskT"] = np.ascontiguousarray(mask.T).astype(BF)
        in_maps.append(m)

    res = run_bass_kernel_spmd(nc, in_maps, core_ids=list(range(NCORES)),
                               trace=TRACE[0])
    LAST_EXEC_NS[0] = res.exec_time_ns
    LAST_RES[0] = res

    acc = np.zeros((NK, 128, E), dtype=np.float32)
    for c in range(NCORES):
        acc += res.results[c]["outp"].astype(np.float32)
    out = acc.reshape(S, E)
    return out[None, :, :]
